# revision 11
# baseline (speedup 1.0000x reference)
"""Trainium2 Bass kernel for nn_BasicBlockBit (ResNet BasicBlock, ternary convs).

Math (per reference):
    out = silu(bn2(conv3x3(silu(bn1(conv3x3(x, q(w1)) + b1)), q(w2)) + b2) + x)
with q() = BitNet ternary quantization (per-tensor median scale).

Strategy:
  - Pure data parallelism: batch 32 -> 4 images per core across 8 cores.
  - Host: quantize weights to exact ternary, fold quant scale + conv bias +
    batchnorm into per-channel scale/bias vectors. Ship x padded (114x114,
    flattened with a 1-element guard) in BOTH bf16 and fp8e4m3.
  - Device, per conv 3x3: process 456 contiguous padded positions per PSUM
    tile (4 padded rows; pad-ring positions compute garbage that is never
    stored). P of the 9 taps run as fp8e4m3 DoubleRow pairs (2 taps per
    matmul pass, pair selected via a 3D access pattern with arbitrary
    stride), the remaining 9-2P taps as bf16 matmuls. This cuts tensor
    engine passes per conv from 9 to 9-P at a small quantization cost.
  - conv1 epilogue: ACT Silu -> mid bf16 interior; GPSIMD converts to a
    parallel e4m3 copy for conv2's DoubleRow pairs.
  - conv2 epilogue: DVE (psum*scale2)+residual -> bf16, ACT Silu(.+bias2)
    -> f32 staging -> DMA out.
"""

import sys

import numpy as np
import ml_dtypes

try:  # concourse normally resolves via the environment's sitecustomize
    import concourse  # noqa: F401
except ImportError:  # pragma: no cover
    sys.path.insert(0, "/opt/trn_rl_repo")

C = 128
H = W = 112
HP = WP = 114  # zero-padded
FLAT = HP * WP  # 12996
GUARD = 1
TL = 13000     # tile/DRAM length per channel: guard + FLAT + tail pad
NPC = 4        # images per core
NCORES = 8
RB = 4         # padded rows per PSUM tile
NPIX = RB * WP  # 456
NBLK = H // RB  # 28
BN_EPS = 1e-5

# 3x3 tap offsets in padded-flat coordinates, sorted
TAPS9 = [dy * WP + dx for dy in (-1, 0, 1) for dx in (-1, 0, 1)]
P1 = 3  # fp8 DoubleRow pairs in conv1 (2*P1 taps fp8, rest bf16)
P2 = 3  # fp8 DoubleRow pairs in conv2

_CACHE = {}


def _pair_view(tile2d, off, step, n):
    """[128, (2, step), (n, 1)] overlapping read view of a 2D SBUF tile."""
    v = tile2d[:, off : off + n].unsqueeze(1)
    v.ap[1] = [step, 2]
    return v


def _dr_matmul(nc, mybir, out, lhsT, rhs, start, stop):
    """fp8 DoubleRow matmul: out += sum_j lhsT[:,j,:].T @ rhs[:,j,:]."""
    eng = nc.tensor
    keep_dims = {0, 1}
    ifmap_ap = eng.lower_ap(rhs.opt(keep_dims), opt=False)
    weights_ap = eng.lower_ap(lhsT.opt(keep_dims), opt=False, for_matmul_weights=True)
    out_ap = eng.lower_ap(out)
    return eng.add_instruction(
        mybir.InstMatmult(
            name=eng.bass.get_next_instruction_name(),
            replication_resolution=0,
            replication_shift_amnt=0,
            replication_num_rows=0,
            start_tensor_calc=start,
            stop_tensor_calc=stop,
            ins=[ifmap_ap, weights_ap],
            outs=[out_ap],
            perf_mode=mybir.MatmulPerfMode.DoubleRow,
            is_transpose=None,
            ifmap_quant_offset=None,
            weights_quant_offset=None,
            bass_skip_group_check=False,
            tile_position=(0, 0),
            tile_size=(128, 128),
        )
    )


def _build_nc():
    import concourse.mybir as mybir
    from concourse import bacc
    from concourse.tile import TileContext

    f32 = mybir.dt.float32
    bf16 = mybir.dt.bfloat16
    fp8 = mybir.dt.float8e4
    Silu = mybir.ActivationFunctionType.Silu
    mult = mybir.AluOpType.mult
    add = mybir.AluOpType.add

    pairs1 = [(TAPS9[2 * i], TAPS9[2 * i + 1]) for i in range(P1)]
    singles1 = TAPS9[2 * P1 :]
    pairs2 = [(TAPS9[2 * i], TAPS9[2 * i + 1]) for i in range(P2)]
    singles2 = TAPS9[2 * P2 :]
    s1n = len(singles1)
    s2n = len(singles2)

    nc = bacc.Bacc(trn_type="TRN2", target_bir_lowering=False, debug=False)

    xb_d = nc.dram_tensor("xb", [NPC, C, TL], bf16, kind="ExternalInput")
    x8_d = nc.dram_tensor("x8", [NPC, C, TL], fp8, kind="ExternalInput")
    wp1_d = nc.dram_tensor("wp1", [C, P1, 2, C], fp8, kind="ExternalInput")
    ws1_d = nc.dram_tensor("ws1", [C, s1n, C], bf16, kind="ExternalInput")
    wp2_d = nc.dram_tensor("wp2", [C, P2, 2, C], fp8, kind="ExternalInput")
    ws2_d = nc.dram_tensor("ws2", [C, s2n, C], bf16, kind="ExternalInput")
    # columns: scale1, bias1, scale2, bias2
    vecs_d = nc.dram_tensor("vecs", [C, 4], f32, kind="ExternalInput")
    # bf16 output store halves the store traffic; host casts back to f32
    out_d = nc.dram_tensor("out", [NPC, C, H * W], bf16, kind="ExternalOutput")

    GS = 4  # output store batching (blocks per DMA)

    with TileContext(nc) as tc:
        with (
            tc.tile_pool(name="consts", bufs=1) as consts,
            tc.tile_pool(name="xbpool", bufs=2) as xbpool,
            tc.tile_pool(name="x8pool", bufs=2) as x8pool,
            tc.tile_pool(name="mbpool", bufs=2) as mbpool,
            tc.tile_pool(name="m8pool", bufs=2) as m8pool,
            tc.tile_pool(name="pspool", bufs=8, space="PSUM") as pspool,
            tc.tile_pool(name="t1pool", bufs=4) as t1pool,
            tc.tile_pool(name="stpool", bufs=3) as stpool,
        ):
            wp1 = consts.tile([C, P1, 2, C], fp8, name="wp1", tag="wp1")
            ws1 = consts.tile([C, s1n, C], bf16, name="ws1", tag="ws1")
            wp2 = consts.tile([C, P2, 2, C], fp8, name="wp2", tag="wp2")
            ws2 = consts.tile([C, s2n, C], bf16, name="ws2", tag="ws2")
            vecs = consts.tile([C, 4], f32, name="vecs", tag="vecs")

            # First image's leading rows + conv1 weights go first so the PE
            # can start as early as possible.
            xb0 = xbpool.tile([C, TL], bf16, name="xb_t", tag="xb")
            x80 = x8pool.tile([C, TL], fp8, name="x8_t", tag="x8")
            c0 = GUARD + 7 * WP
            nc.sync.dma_start(wp1[:, :, :, :], wp1_d.ap())
            nc.sync.dma_start(x80[:, 0:c0], x8_d.ap()[0, :, 0:c0])
            nc.sync.dma_start(ws1[:, :, :], ws1_d.ap())
            nc.sync.dma_start(xb0[:, 0:c0], xb_d.ap()[0, :, 0:c0])
            nc.sync.dma_start(vecs[:, :], vecs_d.ap())
            for a, b in ((c0, 18 * WP), (18 * WP, 57 * WP), (57 * WP, TL)):
                nc.sync.dma_start(x80[:, a:b], x8_d.ap()[0, :, a:b])
                nc.sync.dma_start(xb0[:, a:b], xb_d.ap()[0, :, a:b])
            nc.sync.dma_start(wp2[:, :, :, :], wp2_d.ap())
            nc.sync.dma_start(ws2[:, :, :], ws2_d.ap())

            scale1 = vecs[:, 0:1]
            bias1 = vecs[:, 1:2]
            scale2 = vecs[:, 2:3]
            bias2 = vecs[:, 3:4]

            # Warm the PE HAM clock gate while the first DMAs are in flight.
            # 9 x 512-col cold matmuls ~= 3.8us of PE busy (> the 3.4us HAM
            # SHORT window), so the first real matmuls start at 2.4 GHz.
            warm_sb = consts.tile([C, 512], bf16, name="warm_sb", tag="warm")
            nc.vector.memset(warm_sb[:, :], 0.0)
            warm_ps = pspool.tile([C, 512], f32, name="warm_ps", tag="ps")
            for _ in range(9):
                nc.tensor.matmul(
                    warm_ps[:, :], warm_sb[:, 0:128], warm_sb[:, :],
                    start=True, stop=True,
                )

            for img in range(NPC):
                if img == 0:
                    xb_t, x8_t = xb0, x80
                else:
                    xb_t = xbpool.tile([C, TL], bf16, name="xb_t", tag="xb")
                    x8_t = x8pool.tile([C, TL], fp8, name="x8_t", tag="x8")
                    for a, b in ((0, 6500), (6500, TL)):
                        nc.sync.dma_start(x8_t[:, a:b], x8_d.ap()[img, :, a:b])
                        nc.sync.dma_start(xb_t[:, a:b], xb_d.ap()[img, :, a:b])

                mb_t = mbpool.tile([C, TL], bf16, name="mb_t", tag="mb")
                m8_t = m8pool.tile([C, TL], fp8, name="m8_t", tag="m8")
                mb3 = mb_t[:, GUARD : GUARD + FLAT].rearrange(
                    "p (h w) -> p h w", h=HP
                )
                m83 = m8_t[:, GUARD : GUARD + FLAT].rearrange(
                    "p (h w) -> p h w", h=HP
                )
                xb3 = xb_t[:, GUARD : GUARD + FLAT].rearrange(
                    "p (h w) -> p h w", h=HP
                )
                # zero pad ring (interior gets fully overwritten)
                for t3 in (mb3, m83):
                    nc.vector.memset(t3[:, 0:1, :], 0.0)
                    nc.vector.memset(t3[:, HP - 1 : HP, :], 0.0)
                    nc.vector.memset(t3[:, 1 : HP - 1, 0:1], 0.0)
                    nc.vector.memset(t3[:, 1 : HP - 1, WP - 1 : WP], 0.0)

                # Interleave DR and bf16 passes: a DoubleRow LDWEIGHTS (256
                # cols, ~213ns) cannot fully hide behind one 193ns matmul, so
                # alternating with 128-col bf16 loads keeps the weight-load
                # stream ahead of the matmul stream.
                def emit_conv(ps, wp, ws, pairs, singles, src8, srcb, base):
                    order = []
                    for j in range(max(len(pairs), len(singles))):
                        if j < len(pairs):
                            order.append(("d", j))
                        if j < len(singles):
                            order.append(("s", j))
                    for j, (kind, i) in enumerate(order):
                        start, stop = j == 0, j == len(order) - 1
                        if kind == "d":
                            ta, tb = pairs[i]
                            rhs = _pair_view(src8, base + ta, tb - ta, NPIX)
                            _dr_matmul(nc, mybir, ps[:, :], wp[:, i, :, :],
                                       rhs, start=start, stop=stop)
                        else:
                            t = singles[i]
                            nc.tensor.matmul(
                                ps[:, :], ws[:, i, :],
                                srcb[:, base + t : base + t + NPIX],
                                start=start, stop=stop,
                            )

                # ---- conv1 + bn1 + silu -> mid (bf16 + e4m3) ----
                for blk in range(NBLK):
                    r0 = 1 + RB * blk
                    base = GUARD + r0 * WP
                    ps = pspool.tile([C, NPIX], f32, name="ps", tag="ps")
                    emit_conv(ps, wp1, ws1, pairs1, singles1, x8_t, xb_t, base)
                    ps3 = ps.rearrange("p (r w) -> p r w", r=RB)
                    nc.scalar.activation(
                        mb3[:, r0 : r0 + RB, 1 : 1 + W],
                        ps3[:, :, 1 : 1 + W],
                        Silu, bias=bias1, scale=scale1,
                    )
                    nc.gpsimd.tensor_copy(
                        m83[:, r0 : r0 + RB, 1 : 1 + W],
                        mb3[:, r0 : r0 + RB, 1 : 1 + W],
                    )

                # ---- conv2 + bn2 + residual + silu -> out ----
                st = None
                for blk in range(NBLK):
                    r0 = 1 + RB * blk
                    base = GUARD + r0 * WP
                    ps = pspool.tile([C, NPIX], f32, name="ps", tag="ps")
                    emit_conv(ps, wp2, ws2, pairs2, singles2, m8_t, mb_t, base)
                    ps3 = ps.rearrange("p (r w) -> p r w", r=RB)
                    # t1 = psum*scale2 + x   (bias2 folds into the final ACT)
                    t1 = t1pool.tile([C, RB * W], bf16, name="t1", tag="t1")
                    t13 = t1.rearrange("p (r w) -> p r w", r=RB)
                    nc.vector.scalar_tensor_tensor(
                        t13,
                        ps3[:, :, 1 : 1 + W],
                        scale2,
                        xb3[:, r0 : r0 + RB, 1 : 1 + W],
                        mult, add,
                    )
                    last_group = img == NPC - 1 and blk >= NBLK - GS
                    if last_group:
                        # per-block stores at the very end shorten the tail
                        # chain after the final matmul (parallel DMA queues)
                        st = stpool.tile([C, GS * RB * W], bf16, name="st", tag="st")
                        nc.scalar.activation(
                            st[:, 0 : RB * W], t1[:, :], Silu, bias=bias2
                        )
                        nc.sync.dma_start(
                            out_d.ap()[img, :, blk * RB * W : (blk + 1) * RB * W],
                            st[:, 0 : RB * W],
                        )
                        continue
                    g = blk % GS
                    if g == 0:
                        st = stpool.tile([C, GS * RB * W], bf16, name="st", tag="st")
                    nc.scalar.activation(
                        st[:, g * RB * W : (g + 1) * RB * W],
                        t1[:, :], Silu, bias=bias2,
                    )
                    if g == GS - 1:
                        o0 = (blk - (GS - 1)) * RB * W
                        nc.sync.dma_start(
                            out_d.ap()[img, :, o0 : o0 + GS * RB * W], st[:, :]
                        )

    nc.compile()
    return nc


def _quantize_ternary(w):
    """BitNet ternary quantization, matching the jax reference in fp32."""
    w = np.asarray(w, np.float32)
    scale = np.float32(max(np.float32(np.median(np.abs(w))), np.float32(1e-8)))
    tern = np.clip(np.round(w / scale), -1.0, 1.0).astype(np.float32)
    return tern, scale


def _pack_weights(tern, p):
    """tern [O,I,3,3] -> (pairs [I, p, 2, O] fp8e4, singles [I, s, O] bf16)."""
    # lhsT layout: contraction (cin) on partitions, cout on free dim
    wt = tern.transpose(1, 2, 3, 0).reshape(C, 9, C)  # [cin, tap, cout]
    pairs = np.empty((C, p, 2, C), np.float32)
    for i in range(p):
        pairs[:, i, 0] = wt[:, 2 * i]
        pairs[:, i, 1] = wt[:, 2 * i + 1]
    singles = wt[:, 2 * p :]
    return (
        np.ascontiguousarray(pairs).astype(ml_dtypes.float8_e4m3fn),
        np.ascontiguousarray(singles).astype(ml_dtypes.bfloat16),
    )


def _host_prep(x, w1, b1, g1, be1, m1, v1, w2, b2, g2, be2, m2, v2):
    t1, s1 = _quantize_ternary(w1)
    t2, s2 = _quantize_ternary(w2)
    wp1, ws1 = _pack_weights(t1, P1)
    wp2, ws2 = _pack_weights(t2, P2)
    inv1 = (g1 / np.sqrt(v1 + BN_EPS)).astype(np.float32)
    inv2 = (g2 / np.sqrt(v2 + BN_EPS)).astype(np.float32)
    scale1 = s1 * inv1
    bias1 = b1 * inv1 + be1 - m1 * inv1
    scale2 = s2 * inv2
    bias2 = b2 * inv2 + be2 - m2 * inv2
    vecs = np.stack([scale1, bias1, scale2, bias2], axis=1).astype(np.float32)

    n = x.shape[0]
    xpad = np.zeros((n, C, HP, WP), np.float32)
    xpad[:, :, 1 : 1 + H, 1 : 1 + W] = x
    xb = np.zeros((n, C, TL), dtype=ml_dtypes.bfloat16)
    x8 = np.zeros((n, C, TL), dtype=ml_dtypes.float8_e4m3fn)
    flat = xpad.reshape(n, C, FLAT)
    xb[:, :, GUARD : GUARD + FLAT] = flat.astype(ml_dtypes.bfloat16)
    x8[:, :, GUARD : GUARD + FLAT] = flat.astype(ml_dtypes.float8_e4m3fn)
    return xb, x8, wp1, ws1, wp2, ws2, vecs


def kernel(
    x,
    w1,
    b1,
    bn1_gamma,
    bn1_beta,
    bn1_mean,
    bn1_var,
    w2,
    b2,
    bn2_gamma,
    bn2_beta,
    bn2_mean,
    bn2_var,
    _trace=False,
):
    from concourse.bass_utils import run_bass_kernel_spmd

    x = np.asarray(x, np.float32)
    w1, b1, w2, b2 = (np.asarray(a, np.float32) for a in (w1, b1, w2, b2))
    bn1_gamma, bn1_beta, bn1_mean, bn1_var = (
        np.asarray(a, np.float32) for a in (bn1_gamma, bn1_beta, bn1_mean, bn1_var)
    )
    bn2_gamma, bn2_beta, bn2_mean, bn2_var = (
        np.asarray(a, np.float32) for a in (bn2_gamma, bn2_beta, bn2_mean, bn2_var)
    )

    xb, x8, wp1, ws1, wp2, ws2, vecs = _host_prep(
        x, w1, b1, bn1_gamma, bn1_beta, bn1_mean, bn1_var,
        w2, b2, bn2_gamma, bn2_beta, bn2_mean, bn2_var,
    )

    if "nc" not in _CACHE:
        _CACHE["nc"] = _build_nc()
    nc = _CACHE["nc"]

    in_maps = [
        {
            "xb": np.ascontiguousarray(xb[i * NPC : (i + 1) * NPC]),
            "x8": np.ascontiguousarray(x8[i * NPC : (i + 1) * NPC]),
            "wp1": wp1,
            "ws1": ws1,
            "wp2": wp2,
            "ws2": ws2,
            "vecs": vecs,
        }
        for i in range(NCORES)
    ]
    res = run_bass_kernel_spmd(nc, in_maps, core_ids=list(range(NCORES)), trace=_trace)
    outs = [
        res.results[i]["out"].astype(np.float32).reshape(NPC, C, H, W)
        for i in range(NCORES)
    ]
    full = np.concatenate(outs, axis=0)
    if _trace:
        _CACHE["last_results"] = res
    return full
